# revision 22
# baseline (speedup 1.0000x reference)
"""BitColumnParallelLinear kernel for 8 Trainium2 NeuronCores.

y = x @ sign(W)^T + b, x:[4,2048,4096] f32, W:[16384,4096] f32, b:[16384] f32.

Column-parallel: W rows (out_features) and b sharded 8 ways; x replicated
(uploaded in transposed layout [K, T] as part of the sharding/layout prep);
each core computes its [8192, 2048] output slice on device (sign, f32->f16
cast, matmul, bias add); host concatenates the slices along features.

Per-core device program (fp16 compute, fp32 accumulate):
  - W shard [2048,4096] -> Sign (ACT, fp16 out, sign(0)=0) -> PE transpose ->
    resident SBUF tile [128, 32, 2048] fp16 (k on partitions).
  - b shard -> SBUF row -> partition_broadcast to [128, 2048] f32.
  - x^T streamed per 128-token block: SWDGE cast-DMA f32->fp16 (RNE) into
    [128, 32, 128] k-major tiles, then 32x4 matmuls accumulating into 4 PSUM
    banks (out free dim 512), DVE copyback fused with bias add, DMA out.
"""

import sys

sys.path.insert(0, "/opt/trn_rl_repo")

import numpy as np

T, K, OFULL = 8192, 4096, 16384
NCORES = 8
O = OFULL // NCORES  # 2048 out features per core
P = 128
KT = K // P          # 32 k-tiles
TBLKS = T // P       # 64 token blocks
NFREE = 512
OT = O // NFREE      # 4 out tiles per block

_prog_cache = {}


def build_program():
    if "nc" in _prog_cache:
        return _prog_cache["nc"]
    import concourse.bacc as bacc
    import concourse.mybir as mybir
    import concourse.tile as tile
    from concourse.masks import make_identity

    f32 = mybir.dt.float32
    f16 = mybir.dt.float16

    nc = bacc.Bacc(trn_type="TRN2", dynamic_dma_scratch_size=8192)
    # x delivered as fp16 in tile layout: xt[p, tb, ko*128 + t'] = x[tb*128+t', ko*128+p]
    xt_d = nc.dram_tensor("xt", [P, TBLKS, K], f16, kind="ExternalInput")
    w = nc.dram_tensor("w", [O, K], f32, kind="ExternalInput")
    b = nc.dram_tensor("b", [O], f32, kind="ExternalInput")
    y = nc.dram_tensor("y", [T, O], f32, kind="ExternalOutput")

    HK = K // 2  # W loaded in [128, 2048] halves

    def load_xt(pool, tb):
        xt = pool.tile([P, KT, P], f16, tag="xt", name=f"xt{tb % 4}")
        # one contiguous 8 KiB/partition HWDGE transfer per token block, on the
        # scalar-engine HWDGE queue so W prep owns the sync queue
        nc.scalar.dma_start(xt, xt_d[:, tb, :].rearrange("p (ko t) -> p ko t", t=P))
        return xt

    with tile.TileContext(nc) as tc:
        with tc.tile_pool(name="const", bufs=1) as const, \
             tc.tile_pool(name="wres", bufs=1) as wres, \
             tc.tile_pool(name="wld", bufs=2) as wld, \
             tc.tile_pool(name="ld", bufs=4) as ld, \
             tc.tile_pool(name="tp", bufs=2) as tp, \
             tc.tile_pool(name="outp", bufs=4) as outp, \
             tc.tile_pool(name="psw", bufs=1, space="PSUM") as psw, \
             tc.tile_pool(name="pst", bufs=3, space="PSUM") as pst, \
             tc.tile_pool(name="psm", bufs=4, space="PSUM") as psm:

            ident = const.tile([P, P], f16)
            make_identity(nc, ident)

            # Uninitialized scratch operands for HAM warm-up matmuls; results
            # go to a dedicated PSUM bank and are never read.
            warm_in = const.tile([P, NFREE], f16)
            nc.vector.memset(warm_in, 0.0)
            warm_ps = psw.tile([P, NFREE], f32)
            for _ in range(32):
                nc.tensor.matmul(warm_ps, warm_in[:, :P], warm_in,
                                 start=True, stop=True)

            # Prefetch first x tiles before W prep fills the DMA queues.
            NPRE = 4
            xt_pre = [load_xt(ld, tb) for tb in range(NPRE)]

            # bias broadcast to all partitions: bias_full[p, o] = b[o]
            bias_row = const.tile([1, O], f32)
            nc.sync.dma_start(bias_row, b[None, :])
            bias_full = const.tile([P, O], f32)
            nc.gpsimd.partition_broadcast(bias_full, bias_row)

            # Resident sign(W)^T, k on partitions: wt[p, k, o] = sign(W[o, k*128+p])
            wt = wres.tile([P, KT, O], f16)

            for oc in range(O // P):
                for h in range(2):
                    w_nat = wld.tile([P, HK], f32, tag="wld")
                    nc.sync.dma_start(
                        w_nat, w[oc * P:(oc + 1) * P, h * HK:(h + 1) * HK])
                    w_s = tp.tile([P, HK], f16, tag="tp")
                    nc.scalar.activation(w_s, w_nat,
                                         mybir.ActivationFunctionType.Sign)
                    for kh in range(KT // 2):
                        k = h * (KT // 2) + kh
                        ptr = pst.tile([P, P], f16, tag="tr")
                        nc.tensor.transpose(ptr, w_s[:, kh * P:(kh + 1) * P], ident)
                        nc.vector.tensor_copy(wt[:, k, oc * P:(oc + 1) * P], ptr)
            def mm_group(xt, tb, ot):
                pout = psm.tile([P, NFREE], f32, tag="mm", name=f"mm{ot}")
                for k in range(KT):
                    nc.tensor.matmul(
                        pout,
                        xt[:, k, :],
                        wt[:, k, ot * NFREE:(ot + 1) * NFREE],
                        start=(k == 0),
                        stop=(k == KT - 1),
                    )
                so = outp.tile([P, NFREE], f32, tag="so")
                nc.vector.tensor_tensor(
                    so, pout, bias_full[:, ot * NFREE:(ot + 1) * NFREE],
                    mybir.AluOpType.add)
                nc.scalar.dma_start(
                    y[tb * P:(tb + 1) * P, ot * NFREE:(ot + 1) * NFREE], so)

            # Early phase: ot-outer over the first NPRE token blocks, so the MM
            # stream consumes W-prep chunks in exactly the order they become
            # ready (o-slice ot needs only prep chunks 4ot..4ot+3) — the PE
            # never stalls long enough for the HAM clock gate to re-throttle.
            for ot in range(OT):
                for tb in range(NPRE):
                    mm_group(xt_pre[tb], tb, ot)
            for tb in range(NPRE, TBLKS):
                xt = load_xt(ld, tb)
                for ot in range(OT):
                    mm_group(xt, tb, ot)

    nc.finalize()
    _prog_cache["nc"] = nc
    return nc


def run_on_device(x2d, W, b, core_ids=None, **spmd_kwargs):
    from concourse.bass_utils import run_bass_kernel_spmd

    if core_ids is None:
        core_ids = list(range(NCORES))
    nc = build_program()
    # xt[p, tb, ko*128 + t'] = fp16(x[tb*128 + t', ko*128 + p])
    xt = np.ascontiguousarray(
        x2d.reshape(TBLKS, P, KT, P).transpose(3, 0, 2, 1)
        .astype(np.float16).reshape(P, TBLKS, K))
    in_maps = [
        {"xt": xt,
         "w": np.ascontiguousarray(W[c * O:(c + 1) * O]),
         "b": np.ascontiguousarray(b[c * O:(c + 1) * O])}
        for c in range(NCORES)
    ]
    res = run_bass_kernel_spmd(nc, in_maps, core_ids=core_ids, **spmd_kwargs)
    yfull = np.concatenate([res.results[c]["y"] for c in range(NCORES)], axis=1)
    return yfull, res


def kernel(x, W, b):
    x = np.asarray(x, dtype=np.float32)
    W = np.asarray(W, dtype=np.float32)
    b = np.asarray(b, dtype=np.float32)
    x2d = np.ascontiguousarray(x.reshape(T, K))
    yfull, _ = run_on_device(x2d, W, b)
    return yfull.reshape(x.shape[0], x.shape[1], OFULL).astype(np.float32)


# revision 23
# speedup vs baseline: 1.0071x; 1.0071x over previous
"""BitColumnParallelLinear kernel for 8 Trainium2 NeuronCores.

y = x @ sign(W)^T + b, x:[4,2048,4096] f32, W:[16384,4096] f32, b:[16384] f32.

Column-parallel: W rows (out_features) and b sharded 8 ways; x replicated
(uploaded in transposed layout [K, T] as part of the sharding/layout prep);
each core computes its [8192, 2048] output slice on device (sign, f32->f16
cast, matmul, bias add); host concatenates the slices along features.

Per-core device program (fp16 compute, fp32 accumulate):
  - W shard [2048,4096] -> Sign (ACT, fp16 out, sign(0)=0) -> PE transpose ->
    resident SBUF tile [128, 32, 2048] fp16 (k on partitions).
  - b shard -> SBUF row -> partition_broadcast to [128, 2048] f32.
  - x^T streamed per 128-token block: SWDGE cast-DMA f32->fp16 (RNE) into
    [128, 32, 128] k-major tiles, then 32x4 matmuls accumulating into 4 PSUM
    banks (out free dim 512), DVE copyback fused with bias add, DMA out.
"""

import sys

sys.path.insert(0, "/opt/trn_rl_repo")

import numpy as np

T, K, OFULL = 8192, 4096, 16384
NCORES = 8
O = OFULL // NCORES  # 2048 out features per core
P = 128
KT = K // P          # 32 k-tiles
TBLKS = T // P       # 64 token blocks
NFREE = 512
OT = O // NFREE      # 4 out tiles per block

_prog_cache = {}


def build_program():
    if "nc" in _prog_cache:
        return _prog_cache["nc"]
    import concourse.bacc as bacc
    import concourse.mybir as mybir
    import concourse.tile as tile
    from concourse.masks import make_identity

    f32 = mybir.dt.float32
    f16 = mybir.dt.float16

    nc = bacc.Bacc(trn_type="TRN2", dynamic_dma_scratch_size=8192)
    # x delivered as fp16 in tile layout: xt[p, tb, ko*128 + t'] = x[tb*128+t', ko*128+p]
    xt_d = nc.dram_tensor("xt", [P, TBLKS, K], f16, kind="ExternalInput")
    w = nc.dram_tensor("w", [O, K], f32, kind="ExternalInput")
    b = nc.dram_tensor("b", [O], f32, kind="ExternalInput")
    y = nc.dram_tensor("y", [T, O], f32, kind="ExternalOutput")

    HK = K // 2  # W loaded in [128, 2048] halves

    def load_xt(pool, tb):
        xt = pool.tile([P, KT, P], f16, tag="xt", name=f"xt{tb % 4}")
        # one contiguous 8 KiB/partition HWDGE transfer per token block, on the
        # scalar-engine HWDGE queue so W prep owns the sync queue
        nc.scalar.dma_start(xt, xt_d[:, tb, :].rearrange("p (ko t) -> p ko t", t=P))
        return xt

    with tile.TileContext(nc) as tc:
        with tc.tile_pool(name="const", bufs=1) as const, \
             tc.tile_pool(name="wres", bufs=1) as wres, \
             tc.tile_pool(name="wld", bufs=2) as wld, \
             tc.tile_pool(name="ld", bufs=4) as ld, \
             tc.tile_pool(name="tp", bufs=2) as tp, \
             tc.tile_pool(name="outp", bufs=4) as outp, \
             tc.tile_pool(name="psw", bufs=1, space="PSUM") as psw, \
             tc.tile_pool(name="pst", bufs=3, space="PSUM") as pst, \
             tc.tile_pool(name="psm", bufs=4, space="PSUM") as psm:

            ident = const.tile([P, P], f16)
            make_identity(nc, ident)

            # Uninitialized scratch operands for HAM warm-up matmuls; results
            # go to a dedicated PSUM bank and are never read.
            warm_in = const.tile([P, NFREE], f16)
            nc.vector.memset(warm_in, 0.0)
            warm_ps = psw.tile([P, NFREE], f32)
            for _ in range(32):
                nc.tensor.matmul(warm_ps, warm_in[:, :P], warm_in,
                                 start=True, stop=True)

            # Prefetch first x tiles before W prep fills the DMA queues.
            NPRE = 4
            xt_pre = [load_xt(ld, tb) for tb in range(NPRE)]

            # bias broadcast to all partitions: bias_full[p, o] = b[o]
            bias_row = const.tile([1, O], f32)
            nc.sync.dma_start(bias_row, b[None, :])
            bias_full = const.tile([P, O], f32)
            nc.gpsimd.partition_broadcast(bias_full, bias_row)

            # Resident sign(W)^T, k on partitions: wt[p, k, o] = sign(W[o, k*128+p])
            wt = wres.tile([P, KT, O], f16)

            for oc in range(O // P):
                for h in range(2):
                    w_nat = wld.tile([P, HK], f32, tag="wld")
                    eng = nc.sync if h == 0 else nc.scalar
                    eng.dma_start(
                        w_nat, w[oc * P:(oc + 1) * P, h * HK:(h + 1) * HK])
                    w_s = tp.tile([P, HK], f16, tag="tp")
                    nc.scalar.activation(w_s, w_nat,
                                         mybir.ActivationFunctionType.Sign)
                    for kh in range(KT // 2):
                        k = h * (KT // 2) + kh
                        ptr = pst.tile([P, P], f16, tag="tr")
                        nc.tensor.transpose(ptr, w_s[:, kh * P:(kh + 1) * P], ident)
                        nc.vector.tensor_copy(wt[:, k, oc * P:(oc + 1) * P], ptr)
            def mm_group(xt, tb, ot):
                pout = psm.tile([P, NFREE], f32, tag="mm", name=f"mm{ot}")
                for k in range(KT):
                    nc.tensor.matmul(
                        pout,
                        xt[:, k, :],
                        wt[:, k, ot * NFREE:(ot + 1) * NFREE],
                        start=(k == 0),
                        stop=(k == KT - 1),
                    )
                so = outp.tile([P, NFREE], f32, tag="so")
                nc.vector.tensor_tensor(
                    so, pout, bias_full[:, ot * NFREE:(ot + 1) * NFREE],
                    mybir.AluOpType.add)
                nc.scalar.dma_start(
                    y[tb * P:(tb + 1) * P, ot * NFREE:(ot + 1) * NFREE], so)

            # Early phase: ot-outer over the first NPRE token blocks, so the MM
            # stream consumes W-prep chunks in exactly the order they become
            # ready (o-slice ot needs only prep chunks 4ot..4ot+3) — the PE
            # never stalls long enough for the HAM clock gate to re-throttle.
            for ot in range(OT):
                for tb in range(NPRE):
                    mm_group(xt_pre[tb], tb, ot)
            for tb in range(NPRE, TBLKS):
                xt = load_xt(ld, tb)
                for ot in range(OT):
                    mm_group(xt, tb, ot)

    nc.finalize()
    _prog_cache["nc"] = nc
    return nc


def run_on_device(x2d, W, b, core_ids=None, **spmd_kwargs):
    from concourse.bass_utils import run_bass_kernel_spmd

    if core_ids is None:
        core_ids = list(range(NCORES))
    nc = build_program()
    # xt[p, tb, ko*128 + t'] = fp16(x[tb*128 + t', ko*128 + p])
    xt = np.ascontiguousarray(
        x2d.reshape(TBLKS, P, KT, P).transpose(3, 0, 2, 1)
        .astype(np.float16).reshape(P, TBLKS, K))
    in_maps = [
        {"xt": xt,
         "w": np.ascontiguousarray(W[c * O:(c + 1) * O]),
         "b": np.ascontiguousarray(b[c * O:(c + 1) * O])}
        for c in range(NCORES)
    ]
    res = run_bass_kernel_spmd(nc, in_maps, core_ids=core_ids, **spmd_kwargs)
    yfull = np.concatenate([res.results[c]["y"] for c in range(NCORES)], axis=1)
    return yfull, res


def kernel(x, W, b):
    x = np.asarray(x, dtype=np.float32)
    W = np.asarray(W, dtype=np.float32)
    b = np.asarray(b, dtype=np.float32)
    x2d = np.ascontiguousarray(x.reshape(T, K))
    yfull, _ = run_on_device(x2d, W, b)
    return yfull.reshape(x.shape[0], x.shape[1], OFULL).astype(np.float32)


# revision 24
# speedup vs baseline: 1.0099x; 1.0027x over previous
"""BitColumnParallelLinear kernel for 8 Trainium2 NeuronCores.

y = x @ sign(W)^T + b, x:[4,2048,4096] f32, W:[16384,4096] f32, b:[16384] f32.

Column-parallel: W rows (out_features) and b sharded 8 ways; x replicated
(uploaded in transposed layout [K, T] as part of the sharding/layout prep);
each core computes its [8192, 2048] output slice on device (sign, f32->f16
cast, matmul, bias add); host concatenates the slices along features.

Per-core device program (fp16 compute, fp32 accumulate):
  - W shard [2048,4096] -> Sign (ACT, fp16 out, sign(0)=0) -> PE transpose ->
    resident SBUF tile [128, 32, 2048] fp16 (k on partitions).
  - b shard -> SBUF row -> partition_broadcast to [128, 2048] f32.
  - x^T streamed per 128-token block: SWDGE cast-DMA f32->fp16 (RNE) into
    [128, 32, 128] k-major tiles, then 32x4 matmuls accumulating into 4 PSUM
    banks (out free dim 512), DVE copyback fused with bias add, DMA out.
"""

import sys

sys.path.insert(0, "/opt/trn_rl_repo")

import numpy as np

T, K, OFULL = 8192, 4096, 16384
NCORES = 8
O = OFULL // NCORES  # 2048 out features per core
P = 128
KT = K // P          # 32 k-tiles
TBLKS = T // P       # 64 token blocks
NFREE = 512
OT = O // NFREE      # 4 out tiles per block

_prog_cache = {}


def build_program():
    if "nc" in _prog_cache:
        return _prog_cache["nc"]
    import concourse.bacc as bacc
    import concourse.mybir as mybir
    import concourse.tile as tile
    from concourse.masks import make_identity

    f32 = mybir.dt.float32
    f16 = mybir.dt.float16

    nc = bacc.Bacc(trn_type="TRN2", dynamic_dma_scratch_size=8192)
    # x delivered as fp16 in tile layout: xt[p, tb, ko*128 + t'] = x[tb*128+t', ko*128+p]
    xt_d = nc.dram_tensor("xt", [P, TBLKS, K], f16, kind="ExternalInput")
    w = nc.dram_tensor("w", [O, K], f32, kind="ExternalInput")
    b = nc.dram_tensor("b", [O], f32, kind="ExternalInput")
    y = nc.dram_tensor("y", [T, O], f32, kind="ExternalOutput")

    HK = K // 2  # W loaded in [128, 2048] halves

    def load_xt(pool, tb):
        xt = pool.tile([P, KT, P], f16, tag="xt", name=f"xt{tb % 4}")
        # one contiguous 8 KiB/partition HWDGE transfer per token block, on the
        # scalar-engine HWDGE queue so W prep owns the sync queue
        nc.scalar.dma_start(xt, xt_d[:, tb, :].rearrange("p (ko t) -> p ko t", t=P))
        return xt

    with tile.TileContext(nc) as tc:
        with tc.tile_pool(name="const", bufs=1) as const, \
             tc.tile_pool(name="wres", bufs=1) as wres, \
             tc.tile_pool(name="wld", bufs=2) as wld, \
             tc.tile_pool(name="ld", bufs=4) as ld, \
             tc.tile_pool(name="tp", bufs=2) as tp, \
             tc.tile_pool(name="outp", bufs=4) as outp, \
             tc.tile_pool(name="psw", bufs=1, space="PSUM") as psw, \
             tc.tile_pool(name="pst", bufs=3, space="PSUM") as pst, \
             tc.tile_pool(name="psm", bufs=4, space="PSUM") as psm:

            ident = const.tile([P, P], f16)
            make_identity(nc, ident)

            # Uninitialized scratch operands for HAM warm-up matmuls; results
            # go to a dedicated PSUM bank and are never read.
            warm_in = const.tile([P, NFREE], f16)
            nc.vector.memset(warm_in, 0.0)
            warm_ps = psw.tile([P, NFREE], f32)
            for _ in range(16):
                nc.tensor.matmul(warm_ps, warm_in[:, :P], warm_in,
                                 start=True, stop=True)

            # Prefetch first x tiles before W prep fills the DMA queues.
            NPRE = 4
            xt_pre = [load_xt(ld, tb) for tb in range(NPRE)]

            # bias broadcast to all partitions: bias_full[p, o] = b[o]
            bias_row = const.tile([1, O], f32)
            nc.sync.dma_start(bias_row, b[None, :])
            bias_full = const.tile([P, O], f32)
            nc.gpsimd.partition_broadcast(bias_full, bias_row)

            # Resident sign(W)^T, k on partitions: wt[p, k, o] = sign(W[o, k*128+p])
            wt = wres.tile([P, KT, O], f16)

            for oc in range(O // P):
                for h in range(2):
                    w_nat = wld.tile([P, HK], f32, tag="wld")
                    eng = nc.sync if h == 0 else nc.scalar
                    eng.dma_start(
                        w_nat, w[oc * P:(oc + 1) * P, h * HK:(h + 1) * HK])
                    w_s = tp.tile([P, HK], f16, tag="tp")
                    nc.scalar.activation(w_s, w_nat,
                                         mybir.ActivationFunctionType.Sign)
                    for kh in range(KT // 2):
                        k = h * (KT // 2) + kh
                        ptr = pst.tile([P, P], f16, tag="tr")
                        nc.tensor.transpose(ptr, w_s[:, kh * P:(kh + 1) * P], ident)
                        nc.vector.tensor_copy(wt[:, k, oc * P:(oc + 1) * P], ptr)
            def mm_group(xt, tb, ot):
                pout = psm.tile([P, NFREE], f32, tag="mm", name=f"mm{ot}")
                for k in range(KT):
                    nc.tensor.matmul(
                        pout,
                        xt[:, k, :],
                        wt[:, k, ot * NFREE:(ot + 1) * NFREE],
                        start=(k == 0),
                        stop=(k == KT - 1),
                    )
                so = outp.tile([P, NFREE], f32, tag="so")
                nc.vector.tensor_tensor(
                    so, pout, bias_full[:, ot * NFREE:(ot + 1) * NFREE],
                    mybir.AluOpType.add)
                nc.scalar.dma_start(
                    y[tb * P:(tb + 1) * P, ot * NFREE:(ot + 1) * NFREE], so)

            # Early phase: ot-outer over the first NPRE token blocks, so the MM
            # stream consumes W-prep chunks in exactly the order they become
            # ready (o-slice ot needs only prep chunks 4ot..4ot+3) — the PE
            # never stalls long enough for the HAM clock gate to re-throttle.
            for ot in range(OT):
                for tb in range(NPRE):
                    mm_group(xt_pre[tb], tb, ot)
            for tb in range(NPRE, TBLKS):
                xt = load_xt(ld, tb)
                for ot in range(OT):
                    mm_group(xt, tb, ot)

    nc.finalize()
    _prog_cache["nc"] = nc
    return nc


def run_on_device(x2d, W, b, core_ids=None, **spmd_kwargs):
    from concourse.bass_utils import run_bass_kernel_spmd

    if core_ids is None:
        core_ids = list(range(NCORES))
    nc = build_program()
    # xt[p, tb, ko*128 + t'] = fp16(x[tb*128 + t', ko*128 + p])
    xt = np.ascontiguousarray(
        x2d.reshape(TBLKS, P, KT, P).transpose(3, 0, 2, 1)
        .astype(np.float16).reshape(P, TBLKS, K))
    in_maps = [
        {"xt": xt,
         "w": np.ascontiguousarray(W[c * O:(c + 1) * O]),
         "b": np.ascontiguousarray(b[c * O:(c + 1) * O])}
        for c in range(NCORES)
    ]
    res = run_bass_kernel_spmd(nc, in_maps, core_ids=core_ids, **spmd_kwargs)
    yfull = np.concatenate([res.results[c]["y"] for c in range(NCORES)], axis=1)
    return yfull, res


def kernel(x, W, b):
    x = np.asarray(x, dtype=np.float32)
    W = np.asarray(W, dtype=np.float32)
    b = np.asarray(b, dtype=np.float32)
    x2d = np.ascontiguousarray(x.reshape(T, K))
    yfull, _ = run_on_device(x2d, W, b)
    return yfull.reshape(x.shape[0], x.shape[1], OFULL).astype(np.float32)
